# revision 14
# baseline (speedup 1.0000x reference)
"""Trainium2 Bass kernel for nn_BiGlobal_MPCMFuse (8 NeuronCores, SPMD).

Math (see reference):
    pcm_s  = min over 4 direction pairs of (cen[x+v]-cen[x])*(cen[x-v]-cen[x]),
             v in {(s,s),(s,0),(s,-s),(0,s)}, circular shifts, s in {13, 17}
    pcmN   = BN(pcm_s)  (train-mode BN over (B,H,W) per channel)
    wei    = SE-attention on the (H,W)-pooled pcmN  -> per-(b,c) sigmoid weights
    out    = td_wei * pcm13N + bu_wei * pcm17N

Key identity: with D_v[y] = cen[y+v] - cen[y],
    (cen[x+v]-cen[x]) * (cen[x-v]-cen[x]) = -D_v[x] * D_v[x-v]
so pcm_s = -max_v(D_v[x] * D_v[x-v]) and each direction needs ONE
subtraction (over a slightly extended region) instead of two.
Micro-benchmarks: bf16 tensor_tensor always runs in DVE 2x mode (no
alignment penalty); GpSimd strictly blocks DVE (unusable); ScalarE and
PE run fully concurrent with DVE.

Engine split (pass A):
  - DVE (bottleneck): all muls + max-trees (2x bf16) + the scale-13 subs.
  - PE + ScalarE: the scale-17 subs (and (13,13) on half the chunks) as
    identity-matmul pairs  PSUM = I*cen[y+v] + (-I)*cen[y]  evicted
    PSUM->SBUF bf16 by ScalarE Copy.  This moves ~5.8 of 24 DVE work
    units to otherwise-idle engines.
  - ScalarE also accumulates per-partition sum/sumsq of M = max_v(...)
    (pcm = -M; signs folded on host).

Device strategy (2 NEFF launches + tiny host glue):
  - Shard planes: core k owns 32 planes (b,c); partitions = 32 planes x 4
    row-quarters of 96 rows (+17-row, +17-col circular halos).
  - Pass A over 4 column-chunks of 96; host combines stats, runs BN + SE
    exactly in float64, folds everything into per-plane coefficients
    out = A13*pcm13 + A17*pcm17 + D = (-A13)*M13 + (-A17)*M17 + D.
  - Pass B: per-plane affine combine (ScalarE affine + DVE
    scalar_tensor_tensor), bf16 output upcast to f32 on host (the 2e-2
    rel-err budget dwarfs bf16 rounding).
"""

import os
import sys

import numpy as np

for _p in ("/opt/trn_rl_repo",):
    if _p not in sys.path and os.path.isdir(_p):
        sys.path.insert(0, _p)

import ml_dtypes  # noqa: E402

BF16 = ml_dtypes.bfloat16

B, C, H, W = 4, 64, 384, 384
IC = C // 2
NCORES = 8
P = B * C                  # 256 planes
PPC = P // NCORES          # 32 planes per core
NQ = 4                     # row-quarters per plane
QR = H // NQ               # 96 rows per quarter
SMAX = 17
RHALO = SMAX               # halo rows each side
CHALO = SMAX               # halo cols each side
SR = QR + 2 * RHALO        # 130 stored rows per partition
SW = W + 2 * CHALO         # 418 stored cols
CW = 96                    # column chunk width (pass A); 4 chunks
NCH = W // CW
CR_B = 96                  # rows per pass-B step (whole chunk)
EPS = 1e-5
SCALES = (13, 17)
SSTEP = 2                  # sumsq row-subsample step (sum stays exact:
                           # the SE batch-BN runs over 4 pooled values
                           # ~1% apart, so pooled means must be exact)
PSUM_F32 = 512             # f32 slots per PSUM bank
DMAX = QR + SMAX           # D tile rows (113)
DCMAX = CW + SMAX          # D tile cols (113)

_cache = {}
USE_TTR_FUSE = os.environ.get("TTR", "0") == "1"        # fuse sum(M) into the final max via TTR
STATS_SCRATCH = True


def _dirs(s):
    # v = (vy, vx): the pair product is -D_v[x] * D_v[x-v].
    # Order muls so PE-fed directions come last (their D arrives via
    # PSUM eviction while the DVE works on the earlier directions).
    return ((s, 0), (s, -s), (0, s), (s, s))


def _pe_routed(s, v, ch):
    if s == 17:
        return True
    return v == (13, 13) and ch % 2 == 0


def _build_pass_a():
    import concourse.bacc as bacc
    import concourse.tile as tile
    from concourse import mybir

    nc = bacc.Bacc()
    bf = mybir.dt.bfloat16
    f32 = mybir.dt.float32

    # cenQ[ch, q, r, c] = chunk-major cen: plane/quarter q, quarter-local
    # row r-RHALO, chunk-rel col c-CHALO (circular).  Chunk-major with
    # duplicated halo cols so every DMA is fully contiguous.
    cwf = CW + 2 * CHALO
    cenQ = nc.declare_dram_parameter("cenQ", [NCH, 128, SR, cwf], bf,
                                     isOutput=False)
    eyes = nc.declare_dram_parameter("eyes", [2, 128, 128], bf, isOutput=False)
    m13 = nc.declare_dram_parameter("m13", [NCH, 128, QR, CW], bf, isOutput=True)
    m17 = nc.declare_dram_parameter("m17", [NCH, 128, QR, CW], bf, isOutput=True)
    # per partition: [sum13, sq13, sum17, sq17] x NCH chunks
    stats = nc.declare_dram_parameter("stats", [128, 4 * NCH], f32, isOutput=True)
    m_out = {13: m13, 17: m17}

    with tile.TileContext(nc) as tc:
        with (
            tc.tile_pool(name="eye", bufs=1) as eyep,
            tc.tile_pool(name="cen", bufs=2) as cen_pool,
            tc.tile_pool(name="dve_d", bufs=1) as dbuf,
            tc.tile_pool(name="pe_d", bufs=2) as dpe_buf,
            tc.tile_pool(name="pbuf", bufs=1) as pbuf,
            tc.tile_pool(name="mbuf", bufs=2) as mbuf,
            tc.tile_pool(name="accp", bufs=1) as accp,
            tc.tile_pool(name="psum", bufs=2, space="PSUM") as psum_pool,
        ):
            eyeP = eyep.tile([128, 128], bf, tag="eyeP", name="eyeP")
            eyeN = eyep.tile([128, 128], bf, tag="eyeN", name="eyeN")
            nc.sync.dma_start(out=eyeP, in_=eyes[0])
            nc.sync.dma_start(out=eyeN, in_=eyes[1])
            acc = accp.tile([128, 4 * NCH], f32, tag="acc", name="acc")
            cen_tiles = {}
            pe_tiles = {}
            dve_scr = [None]
            pe_scr = [None]

            def d_region(s, vy, vx):
                r0d = -vy
                nrd = QR + vy
                c0d = -s if vx > 0 else 0
                ncd = CW + (s if vx != 0 else 0)
                return r0d, nrd, c0d, ncd

            def cen_at(cen, r0, c0l, nr, ncl):
                # cen AP for local rows [r0, r0+nr), chunk-rel cols
                # [c0l, c0l+ncl)
                return cen[
                    :, r0 + RHALO : r0 + RHALO + nr,
                    c0l + CHALO : c0l + CHALO + ncl,
                ]

            def pe_work(ch):
                # cen prefetch + the PE-routed subs: identity-matmul pairs
                # -> PSUM, ScalarE eviction PSUM -> SBUF bf16.
                cen = cen_pool.tile([128, SR, cwf], bf, tag="cen",
                                    name=f"cen{ch}")
                nc.sync.dma_start(out=cen, in_=cenQ[ch])
                cen_tiles[ch] = cen
                for s in SCALES:
                    for vy, vx in _dirs(s):
                        if not _pe_routed(s, (vy, vx), ch):
                            continue
                        r0d, nrd, c0d, ncd = d_region(s, vy, vx)
                        D = dpe_buf.tile([128, DMAX, DCMAX], bf, tag="Dpe",
                                         name=f"Dpe{ch}_{s}_{vy}_{vx}")
                        slr = PSUM_F32 // ncd       # rows per PSUM slice
                        gr = 4 * slr                # rows per psum buffer
                        r = 0
                        while r < nrd:
                            g_rows = min(gr, nrd - r)
                            ps = psum_pool.tile([128, 4 * PSUM_F32], f32,
                                                tag="ps", name="ps")
                            nsl = (g_rows + slr - 1) // slr
                            # all eyeP matmuls first, then all eyeN: the
                            # weight matrix only changes once per group, so
                            # 2 LDWEIGHTS instead of 2*nsl.
                            for sl in range(nsl):
                                sr0 = r + sl * slr
                                s_rows = min(slr, nrd - sr0)
                                out_ap = ps[:, sl * PSUM_F32 :
                                            sl * PSUM_F32 + s_rows * ncd]
                                nc.tensor.matmul(
                                    out_ap,
                                    eyeP,
                                    cen_at(cen, r0d + sr0 + vy, c0d + vx,
                                           s_rows, ncd),
                                    start=True, stop=False,
                                )
                            for sl in range(nsl):
                                sr0 = r + sl * slr
                                s_rows = min(slr, nrd - sr0)
                                out_ap = ps[:, sl * PSUM_F32 :
                                            sl * PSUM_F32 + s_rows * ncd]
                                nc.tensor.matmul(
                                    out_ap,
                                    eyeN,
                                    cen_at(cen, r0d + sr0, c0d, s_rows, ncd),
                                    start=False, stop=True,
                                )
                            # one batched eviction for the whole buffer
                            pview = ps.rearrange("p (g x) -> p g x", g=4)
                            if g_rows == gr:
                                nc.scalar.activation(
                                    D[:, r : r + gr, 0:ncd],
                                    pview[:, :, 0 : slr * ncd],
                                    mybir.ActivationFunctionType.Copy,
                                )
                            else:
                                for sl in range(nsl):
                                    sr0 = r + sl * slr
                                    s_rows = min(slr, nrd - sr0)
                                    nc.scalar.activation(
                                        D[:, sr0 : sr0 + s_rows, 0:ncd],
                                        pview[:, sl, 0 : s_rows * ncd],
                                        mybir.ActivationFunctionType.Copy,
                                    )
                            r += g_rows
                        pe_tiles[(ch, s, vy, vx)] = D

            def dve_work(ch):
                # DVE-routed subs + all muls / max-trees + M DMAs; returns
                # the M tiles so stats can be emitted later.  The final max
                # of each scale is a tensor_tensor_reduce whose accum_out
                # produces sum(M) for free (replaces a ScalarE Copy+accum).
                cen = cen_tiles[ch]
                ms = {}
                for si, s in enumerate(SCALES):
                    M = mbuf.tile([128, QR, CW], bf, tag="M", name=f"M{s}_{ch}")
                    sum_slot = acc[:, 4 * ch + 2 * si : 4 * ch + 2 * si + 1]
                    for di, (vy, vx) in enumerate(_dirs(s)):
                        r0d, nrd, c0d, ncd = d_region(s, vy, vx)
                        if _pe_routed(s, (vy, vx), ch):
                            D = pe_tiles.pop((ch, s, vy, vx))
                        else:
                            D = dbuf.tile([128, DMAX, DCMAX], bf, tag="D",
                                          name="D", bufs=1)
                            dve_scr[0] = D
                            # D_v[y] = cen[y+v] - cen[y]
                            nc.vector.tensor_sub(
                                D[:, 0:nrd, 0:ncd],
                                cen_at(cen, r0d + vy, c0d + vx, nrd, ncd),
                                cen_at(cen, r0d, c0d, nrd, ncd),
                            )
                        # P_v[x] = D_v[x] * D_v[x-v], x in [0,QR)x[0,CW)
                        in0 = D[:, -r0d : -r0d + QR, -c0d : -c0d + CW]
                        in1 = D[:, -r0d - vy : -r0d - vy + QR,
                                -c0d - vx : -c0d - vx + CW]
                        if di == 0:
                            nc.vector.tensor_mul(M, in0, in1)
                        else:
                            Pv = pbuf.tile([128, QR, CW], bf, tag="P",
                                           name="P", bufs=1)
                            pe_scr[0] = Pv
                            nc.vector.tensor_mul(Pv, in0, in1)
                            if di == 3 and USE_TTR_FUSE:
                                nc.vector.tensor_tensor_reduce(
                                    out=M, in0=M, in1=Pv, scale=1.0,
                                    scalar=0.0,
                                    op0=mybir.AluOpType.max,
                                    op1=mybir.AluOpType.add,
                                    accum_out=sum_slot,
                                )
                            else:
                                nc.vector.tensor_tensor(
                                    M, M, Pv, op=mybir.AluOpType.max
                                )
                    nc.sync.dma_start(out=m_out[s][ch], in_=M)
                    ms[s] = M
                return ms

            def stats_work(ch, ms, scr13, scr17):
                # pcm = -M; host flips signs.  sum(M) is produced by the
                # final max's tensor_tensor_reduce in dve_work; only the
                # (subsampled) sum-of-squares runs here on ScalarE.
                # Square writes a SCRATCH tile (not M in-place): ready
                # right after the last max instead of after the M DMA, so
                # the in-order ScalarE queue never parks the next chunk's
                # DVE-critical evictions behind idle-waiting stats.
                # sumsq reads every SSTEP-th row: the BN variance pools
                # ~150K samples/channel so sampling noise is far inside
                # the 2e-2 budget (sums stay exact: the SE batch-BN
                # amplifies per-batch mean differences of ~1%, so pooled
                # means cannot be subsampled).
                for si, s in enumerate(SCALES):
                    M = ms[s]
                    scr = scr13 if s == 13 else scr17
                    sum_slot = acc[:, 4 * ch + 2 * si : 4 * ch + 2 * si + 1]
                    sq_slot = acc[:, 4 * ch + 2 * si + 1 :
                                  4 * ch + 2 * si + 2]
                    if not USE_TTR_FUSE:
                        nc.scalar.activation(
                            scr[:, 0:QR, 0:CW], M,
                            mybir.ActivationFunctionType.Copy,
                            accum_out=sum_slot,
                        )
                    nc.scalar.activation(
                        scr[:, 0 : QR // SSTEP, 0:CW],
                        M[:, 0:QR:SSTEP, :],
                        mybir.ActivationFunctionType.Square,
                        accum_out=sq_slot,
                    )

            for ch in range(NCH):
                pe_work(ch)
                ms = dve_work(ch)
                stats_work(ch, ms, dve_scr[0], pe_scr[0])
            nc.sync.dma_start(out=stats.ap(), in_=acc)
    return nc


def _build_pass_b():
    import concourse.bacc as bacc
    import concourse.tile as tile
    from concourse import mybir

    nc = bacc.Bacc()
    bf = mybir.dt.bfloat16
    f32 = mybir.dt.float32

    m13 = nc.declare_dram_parameter("m13", [NCH, 128, QR, CW], bf, isOutput=False)
    m17 = nc.declare_dram_parameter("m17", [NCH, 128, QR, CW], bf, isOutput=False)
    # per partition: [A13', A17', D', pad]  (primes: signs folded on host)
    coef = nc.declare_dram_parameter("coef", [128, 4], f32, isOutput=False)
    out = nc.declare_dram_parameter("out", [NCH, 128, QR, CW], bf, isOutput=True)

    nsteps = QR // CR_B
    with tile.TileContext(nc) as tc:
        with (
            tc.tile_pool(name="cf", bufs=1) as cfp,
            tc.tile_pool(name="io", bufs=2) as io,
        ):
            cf = cfp.tile([128, 4], f32, tag="cf", name="cf")
            nc.sync.dma_start(out=cf, in_=coef.ap())
            for ch in range(NCH):
                for step in range(nsteps):
                    r0 = step * CR_B
                    t13 = io.tile([128, CR_B, CW], bf, tag="t13", name="t13")
                    t17 = io.tile([128, CR_B, CW], bf, tag="t17", name="t17")
                    nc.sync.dma_start(out=t13, in_=m13[ch, :, r0 : r0 + CR_B, :])
                    nc.sync.dma_start(out=t17, in_=m17[ch, :, r0 : r0 + CR_B, :])
                    # u = A13'*m13 + D'  (ScalarE free affine)
                    u = io.tile([128, CR_B, CW], bf, tag="u", name="u")
                    nc.scalar.activation(
                        u, t13, mybir.ActivationFunctionType.Identity,
                        scale=cf[:, 0:1], bias=cf[:, 2:3],
                    )
                    # o = (m17 * A17') + u   (DVE fused scalar_tensor_tensor)
                    o = io.tile([128, CR_B, CW], bf, tag="o", name="o")
                    nc.vector.scalar_tensor_tensor(
                        out=o, in0=t17, scalar=cf[:, 1:2], in1=u,
                        op0=mybir.AluOpType.mult, op1=mybir.AluOpType.add,
                    )
                    nc.sync.dma_start(out=out[ch, :, r0 : r0 + CR_B, :], in_=o)
    return nc


def _build_fused():
    """Single-NEFF kernel: phase-1 pcm compute (as pass A) -> stats
    AllGather across the 8 cores -> on-device BN + SE -> per-plane
    coefficient gather -> apply phase (reload M from DRAM scratch).

    On-device SE numerics: the SE batch-BN normalizes over 4 pooled
    values that differ by ~1%, so the pooled means are CENTERED in f32
    before any bf16 cast (BN(y) is invariant to pre-centering, and the
    centered values tolerate bf16's 0.4% relative error)."""
    import concourse.bacc as bacc
    import concourse.tile as tile
    from concourse import mybir

    nc = bacc.Bacc()
    bf = mybir.dt.bfloat16
    f32 = mybir.dt.float32
    AF = mybir.ActivationFunctionType
    OP = mybir.AluOpType
    AX = mybir.AxisListType

    cwf = CW + 2 * CHALO
    n_el = float(B * H * W)
    n_sq = float(B * H * W // SSTEP)
    hw = float(H * W)

    cenQ = nc.declare_dram_parameter("cenQ", [NCH, 128, SR, cwf], bf,
                                     isOutput=False)
    eyes = nc.declare_dram_parameter("eyes", [2, 128, 128], bf, isOutput=False)
    # SE weights (same on all cores): wse1[2*se+h][l][oc] = w1[oc, l+32h]
    wse1 = nc.declare_dram_parameter("wse1", [4, 32, 32], bf, isOutput=False)
    # wse2[se][oc][c] = w2[c, oc]
    wse2 = nc.declare_dram_parameter("wse2", [2, 32, 64], bf, isOutput=False)
    # whalf[h][l][c] = 1 iff c == l + 32h   (expand [32,2] -> [64])
    whalf = nc.declare_dram_parameter("whalf", [2, 32, 64], f32, isOutput=False)
    # seg1[se][oc] = (gamma1, beta1); seg2[se][c] = (gamma2, beta2)
    seg1 = nc.declare_dram_parameter("seg1", [2, 32, 2], f32, isOutput=False)
    seg2 = nc.declare_dram_parameter("seg2", [2, 64, 2], f32, isOutput=False)
    # bnw[l][h] = (bn1_g, bn1_b, bn2_g, bn2_b) for channel c = l + 32h
    bnw = nc.declare_dram_parameter("bnw", [32, 2, 4], f32, isOutput=False)
    # per-core: sel[c][p] = 1 iff c == c(p); bmask[p][3b+j] = 1 iff b == b(core)
    sel = nc.declare_dram_parameter("sel", [64, 128], f32, isOutput=False)
    bmask = nc.declare_dram_parameter("bmask", [128, 12], f32, isOutput=False)
    out_t = nc.declare_dram_parameter("out", [NCH, 128, QR, CW], bf,
                                      isOutput=True)

    m13d = nc.dram_tensor("m13d", [NCH, 128, QR, CW], bf)
    m17d = nc.dram_tensor("m17d", [NCH, 128, QR, CW], bf)
    statsL = nc.dram_tensor("statsL", [128, 4 * NCH], f32)
    statsG = nc.dram_tensor("statsG", [NCORES, 128, 4 * NCH], f32,
                            addr_space="Shared")
    m_out = {13: m13d, 17: m17d}

    with tile.TileContext(nc) as tc:
        with (
            tc.tile_pool(name="eye", bufs=1) as eyep,
            tc.tile_pool(name="accp", bufs=1) as accp,
            tc.tile_pool(name="sew", bufs=1) as sew,
            tc.tile_pool(name="sm", bufs=1) as sm,
        ):
            eyeP = eyep.tile([128, 128], bf, tag="eyeP", name="eyeP")
            eyeN = eyep.tile([128, 128], bf, tag="eyeN", name="eyeN")
            nc.sync.dma_start(out=eyeP, in_=eyes[0])
            nc.sync.dma_start(out=eyeN, in_=eyes[1])
            acc = accp.tile([128, 4 * NCH], f32, tag="acc", name="acc")

            w1sb = sew.tile([32, 4, 32], bf, tag="w1sb", name="w1sb")
            w2sb = sew.tile([32, 2, 64], bf, tag="w2sb", name="w2sb")
            whsb = sew.tile([32, 2, 64], f32, tag="whsb", name="whsb")
            sg1 = sew.tile([32, 2, 2], f32, tag="sg1", name="sg1")
            sg2 = sew.tile([64, 2, 2], f32, tag="sg2", name="sg2")
            bnwsb = sew.tile([32, 2, 4], f32, tag="bnwsb", name="bnwsb")
            selsb = sew.tile([64, 128], f32, tag="selsb", name="selsb")
            bmsb = sew.tile([128, 12], f32, tag="bmsb", name="bmsb")
            nc.sync.dma_start(out=w1sb, in_=wse1.ap().rearrange("i l o -> l i o"))
            nc.sync.dma_start(out=w2sb, in_=wse2.ap().rearrange("s o c -> o s c"))
            nc.sync.dma_start(out=whsb, in_=whalf.ap().rearrange("h l c -> l h c"))
            nc.sync.dma_start(out=sg1, in_=seg1.ap().rearrange("s o t -> o s t"))
            nc.sync.dma_start(out=sg2, in_=seg2.ap().rearrange("s c t -> c s t"))
            nc.sync.dma_start(out=bnwsb, in_=bnw.ap())
            nc.sync.dma_start(out=selsb, in_=sel.ap())
            nc.sync.dma_start(out=bmsb, in_=bmask.ap())

            # ---------------- phase 1: pcm compute ----------------
            with (
                tc.tile_pool(name="cen", bufs=2) as cen_pool,
                tc.tile_pool(name="dve_d", bufs=1) as dbuf,
                tc.tile_pool(name="pe_d", bufs=2) as dpe_buf,
                tc.tile_pool(name="pbuf", bufs=1) as pbuf,
                tc.tile_pool(name="mbuf", bufs=2) as mbuf,
                tc.tile_pool(name="psumA", bufs=2, space="PSUM") as psum_pool,
            ):
                cen_tiles = {}
                pe_tiles = {}
                dve_scr = [None]
                pe_scr = [None]

                def d_region(s, vy, vx):
                    r0d = -vy
                    nrd = QR + vy
                    c0d = -s if vx > 0 else 0
                    ncd = CW + (s if vx != 0 else 0)
                    return r0d, nrd, c0d, ncd

                def cen_at(cen, r0, c0l, nr, ncl):
                    return cen[
                        :, r0 + RHALO : r0 + RHALO + nr,
                        c0l + CHALO : c0l + CHALO + ncl,
                    ]

                def pe_work(ch):
                    cen = cen_pool.tile([128, SR, cwf], bf, tag="cen",
                                        name=f"cen{ch}")
                    nc.sync.dma_start(out=cen, in_=cenQ[ch])
                    cen_tiles[ch] = cen
                    for s in SCALES:
                        for vy, vx in _dirs(s):
                            if not _pe_routed(s, (vy, vx), ch):
                                continue
                            r0d, nrd, c0d, ncd = d_region(s, vy, vx)
                            D = dpe_buf.tile([128, DMAX, DCMAX], bf, tag="Dpe",
                                             name=f"Dpe{ch}_{s}_{vy}_{vx}")
                            slr = PSUM_F32 // ncd
                            gr = 4 * slr
                            r = 0
                            while r < nrd:
                                g_rows = min(gr, nrd - r)
                                ps = psum_pool.tile([128, 4 * PSUM_F32], f32,
                                                    tag="ps", name="ps")
                                nsl = (g_rows + slr - 1) // slr
                                for sl in range(nsl):
                                    sr0 = r + sl * slr
                                    s_rows = min(slr, nrd - sr0)
                                    out_ap = ps[:, sl * PSUM_F32 :
                                                sl * PSUM_F32 + s_rows * ncd]
                                    nc.tensor.matmul(
                                        out_ap, eyeP,
                                        cen_at(cen, r0d + sr0 + vy, c0d + vx,
                                               s_rows, ncd),
                                        start=True, stop=False,
                                    )
                                for sl in range(nsl):
                                    sr0 = r + sl * slr
                                    s_rows = min(slr, nrd - sr0)
                                    out_ap = ps[:, sl * PSUM_F32 :
                                                sl * PSUM_F32 + s_rows * ncd]
                                    nc.tensor.matmul(
                                        out_ap, eyeN,
                                        cen_at(cen, r0d + sr0, c0d, s_rows, ncd),
                                        start=False, stop=True,
                                    )
                                pview = ps.rearrange("p (g x) -> p g x", g=4)
                                if g_rows == gr:
                                    nc.scalar.activation(
                                        D[:, r : r + gr, 0:ncd],
                                        pview[:, :, 0 : slr * ncd], AF.Copy,
                                    )
                                else:
                                    for sl in range(nsl):
                                        sr0 = r + sl * slr
                                        s_rows = min(slr, nrd - sr0)
                                        nc.scalar.activation(
                                            D[:, sr0 : sr0 + s_rows, 0:ncd],
                                            pview[:, sl, 0 : s_rows * ncd],
                                            AF.Copy,
                                        )
                                r += g_rows
                            pe_tiles[(ch, s, vy, vx)] = D

                def dve_work(ch):
                    cen = cen_tiles[ch]
                    ms = {}
                    for si, s in enumerate(SCALES):
                        M = mbuf.tile([128, QR, CW], bf, tag="M",
                                      name=f"M{s}_{ch}")
                        for di, (vy, vx) in enumerate(_dirs(s)):
                            r0d, nrd, c0d, ncd = d_region(s, vy, vx)
                            if _pe_routed(s, (vy, vx), ch):
                                D = pe_tiles.pop((ch, s, vy, vx))
                            else:
                                D = dbuf.tile([128, DMAX, DCMAX], bf, tag="D",
                                              name="D", bufs=1)
                                dve_scr[0] = D
                                nc.vector.tensor_sub(
                                    D[:, 0:nrd, 0:ncd],
                                    cen_at(cen, r0d + vy, c0d + vx, nrd, ncd),
                                    cen_at(cen, r0d, c0d, nrd, ncd),
                                )
                            in0 = D[:, -r0d : -r0d + QR, -c0d : -c0d + CW]
                            in1 = D[:, -r0d - vy : -r0d - vy + QR,
                                    -c0d - vx : -c0d - vx + CW]
                            if di == 0:
                                nc.vector.tensor_mul(M, in0, in1)
                            else:
                                Pv = pbuf.tile([128, QR, CW], bf, tag="P",
                                               name="P", bufs=1)
                                pe_scr[0] = Pv
                                nc.vector.tensor_mul(Pv, in0, in1)
                                nc.vector.tensor_tensor(
                                    M, M, Pv, op=OP.max
                                )
                        nc.sync.dma_start(out=m_out[s][ch], in_=M)
                        ms[s] = M
                    return ms

                def stats_work(ch, ms, scr13, scr17):
                    for si, s in enumerate(SCALES):
                        M = ms[s]
                        scr = scr13 if s == 13 else scr17
                        sum_slot = acc[:, 4 * ch + 2 * si :
                                       4 * ch + 2 * si + 1]
                        sq_slot = acc[:, 4 * ch + 2 * si + 1 :
                                      4 * ch + 2 * si + 2]
                        nc.scalar.activation(
                            scr[:, 0:QR, 0:CW], M, AF.Copy,
                            accum_out=sum_slot,
                        )
                        nc.scalar.activation(
                            scr[:, 0 : QR // SSTEP, 0:CW],
                            M[:, 0:QR:SSTEP, :], AF.Square,
                            accum_out=sq_slot,
                        )

                for ch in range(NCH):
                    pe_work(ch)
                    ms = dve_work(ch)
                    stats_work(ch, ms, dve_scr[0], pe_scr[0])
                nc.sync.dma_start(out=statsL.ap(), in_=acc)

            # ---------------- stats exchange + BN/SE ----------------
            nc.gpsimd.collective_compute(
                kind="AllGather",
                op=OP.bypass,
                replica_groups=[list(range(NCORES))],
                ins=[statsL.ap()],
                outs=[statsG.ap()],
            )

            # g4[l, k, q, s] <- statsG[k, 4l+q, s]
            g4 = sm.tile([32, NCORES, 4, 4 * NCH], f32, tag="g4", name="g4")
            nc.sync.dma_start(
                out=g4,
                in_=statsG.ap().rearrange("k (l q) s -> l k q s", q=4),
            )
            # reduce chunks: t1[l, (k q), slot]
            t1 = sm.tile([32, 32, 4], f32, tag="t1", name="t1")
            nc.vector.tensor_reduce(
                out=t1,
                in_=g4.rearrange("l k q (ch sl) -> l (k q) sl ch", ch=NCH),
                axis=AX.X, op=OP.add,
            )
            # reduce quarters: T2[l, k, slot]
            T2 = sm.tile([32, NCORES, 4], f32, tag="T2", name="T2")
            nc.vector.tensor_reduce(
                out=T2,
                in_=t1.rearrange("l (k q) sl -> l k sl q", q=4),
                axis=AX.X, op=OP.add,
            )
            # reduce batches (k = 2b + h): S[l, h, slot]
            S = sm.tile([32, 2, 4], f32, tag="S", name="S")
            nc.vector.tensor_reduce(
                out=S,
                in_=T2.rearrange("l (b h) sl -> l h sl b", h=2),
                axis=AX.X, op=OP.add,
            )

            def t32(tag, shape=(32, 2)):
                return sm.tile([shape[0], shape[1]], f32, tag=tag, name=tag)

            def bn_coef(si):
                # a = gamma * rsqrt(var + eps); bp = beta - mean * a
                ssum = S[:, :, 2 * si]
                ssq = S[:, :, 2 * si + 1]
                mean = t32(f"mean{si}")
                nc.vector.tensor_scalar(mean, ssum, -1.0 / n_el, None, OP.mult)
                ex2 = t32(f"ex2{si}")
                nc.vector.tensor_scalar(ex2, ssq, 1.0 / n_sq, None, OP.mult)
                msq = t32(f"msq{si}")
                nc.vector.tensor_mul(msq, mean, mean)
                var = t32(f"var{si}")
                nc.vector.tensor_sub(var, ex2, msq)
                varp = t32(f"varp{si}")
                nc.vector.tensor_scalar(varp, var, EPS, None, OP.add)
                std = t32(f"std{si}")
                nc.scalar.activation(std, varp, AF.Sqrt)
                rstd = t32(f"rstd{si}")
                nc.vector.reciprocal(rstd, std)
                a = t32(f"a{si}")
                nc.vector.tensor_mul(a, rstd, bnwsb[:, :, 2 * si])
                ma = t32(f"ma{si}")
                nc.vector.tensor_mul(ma, mean, a)
                bp = t32(f"bp{si}")
                nc.vector.tensor_sub(bp, bnwsb[:, :, 2 * si + 1], ma)
                return a, bp

            a13, bp13 = bn_coef(0)
            a17, bp17 = bn_coef(1)

            def se_input(si, a):
                # xb[l, h, b] = (-a/HW) * (q - mean_b q),  q = sum(M) per
                # (plane, chunk-summed);  f32-centered, then bf16.
                asc = t32(f"asc{si}")
                nc.vector.tensor_scalar(asc, a, -1.0 / hw, None, OP.mult)
                xb = sm.tile([32, 2, 4], bf, tag=f"xb{si}", name=f"xb{si}")
                for h in (0, 1):
                    q = T2[:, h::2, 2 * si]              # [32, 4(b)]
                    qs = t32(f"qs{si}{h}", (32, 1))
                    nc.vector.tensor_reduce(out=qs, in_=q, axis=AX.X,
                                            op=OP.add)
                    qb4 = t32(f"qb4{si}{h}", (32, 1))
                    nc.vector.tensor_scalar(qb4, qs, 0.25, None, OP.mult)
                    qc = t32(f"qc{si}{h}", (32, 4))
                    nc.vector.tensor_scalar(qc, q, qb4, None, OP.subtract)
                    nc.vector.tensor_scalar(xb[:, h, :], qc,
                                            asc[:, h : h + 1], None, OP.mult)
                return xb

            xb13 = se_input(0, a13)
            xb17 = se_input(1, a17)

            with tc.tile_pool(name="psumB", bufs=1, space="PSUM") as pse:

                def se_block(se, xb):
                    g1c = sg1[:, se, 0:1]
                    be1c = sg1[:, se, 1:2]
                    g2c = sg2[:, se, 0:1]
                    be2c = sg2[:, se, 1:2]
                    psY = pse.tile([32, 4], f32, tag=f"psY{se}",
                                   name=f"psY{se}")
                    nc.tensor.matmul(psY, w1sb[:, 2 * se, :], xb[:, 0, :],
                                     start=True, stop=False)
                    nc.tensor.matmul(psY, w1sb[:, 2 * se + 1, :], xb[:, 1, :],
                                     start=False, stop=True)
                    yb = t32(f"yb{se}", (32, 1))
                    nc.vector.tensor_reduce(out=yb, in_=psY, axis=AX.X,
                                            op=OP.add)
                    yb4 = t32(f"yb4{se}", (32, 1))
                    nc.vector.tensor_scalar(yb4, yb, 0.25, None, OP.mult)
                    yc = t32(f"yc{se}", (32, 4))
                    nc.vector.tensor_scalar(yc, psY, yb4, None, OP.subtract)
                    ysq = t32(f"ysq{se}", (32, 4))
                    nc.vector.tensor_mul(ysq, yc, yc)
                    vs = t32(f"vs{se}", (32, 1))
                    nc.vector.tensor_reduce(out=vs, in_=ysq, axis=AX.X,
                                            op=OP.add)
                    v4 = t32(f"v4{se}", (32, 1))
                    nc.vector.tensor_scalar(v4, vs, 0.25, None, OP.mult)
                    v4e = t32(f"v4e{se}", (32, 1))
                    nc.vector.tensor_scalar(v4e, v4, EPS, None, OP.add)
                    sd = t32(f"sd{se}", (32, 1))
                    nc.scalar.activation(sd, v4e, AF.Sqrt)
                    rs = t32(f"rs{se}", (32, 1))
                    nc.vector.reciprocal(rs, sd)
                    yn = t32(f"yn{se}", (32, 4))
                    nc.vector.tensor_scalar(yn, yc, rs, None, OP.mult)
                    ya = t32(f"ya{se}", (32, 4))
                    nc.vector.tensor_scalar(ya, yn, g1c, be1c, OP.mult, OP.add)
                    yr = t32(f"yr{se}", (32, 4))
                    nc.vector.tensor_relu(yr, ya)
                    rb = sm.tile([32, 4], bf, tag=f"rb{se}", name=f"rb{se}")
                    nc.vector.tensor_copy(rb, yr)
                    psZ = pse.tile([64, 4], f32, tag=f"psZ{se}",
                                   name=f"psZ{se}")
                    nc.tensor.matmul(psZ, w2sb[:, se, :], rb,
                                     start=True, stop=True)
                    zb = t32(f"zb{se}", (64, 1))
                    nc.vector.tensor_reduce(out=zb, in_=psZ, axis=AX.X,
                                            op=OP.add)
                    zb4 = t32(f"zb4{se}", (64, 1))
                    nc.vector.tensor_scalar(zb4, zb, 0.25, None, OP.mult)
                    zc = t32(f"zc{se}", (64, 4))
                    nc.vector.tensor_scalar(zc, psZ, zb4, None, OP.subtract)
                    zsq = t32(f"zsq{se}", (64, 4))
                    nc.vector.tensor_mul(zsq, zc, zc)
                    zv = t32(f"zv{se}", (64, 1))
                    nc.vector.tensor_reduce(out=zv, in_=zsq, axis=AX.X,
                                            op=OP.add)
                    zv4 = t32(f"zv4{se}", (64, 1))
                    nc.vector.tensor_scalar(zv4, zv, 0.25, None, OP.mult)
                    zv4e = t32(f"zv4e{se}", (64, 1))
                    nc.vector.tensor_scalar(zv4e, zv4, EPS, None, OP.add)
                    zsd = t32(f"zsd{se}", (64, 1))
                    nc.scalar.activation(zsd, zv4e, AF.Sqrt)
                    zrs = t32(f"zrs{se}", (64, 1))
                    nc.vector.reciprocal(zrs, zsd)
                    zn = t32(f"zn{se}", (64, 4))
                    nc.vector.tensor_scalar(zn, zc, zrs, None, OP.mult)
                    wei = t32(f"wei{se}", (64, 4))
                    nc.scalar.activation(wei, zn, AF.Sigmoid,
                                         scale=g2c, bias=be2c)
                    return wei

                wei_td = se_block(0, xb17)   # topdown attends pcm17
                wei_bu = se_block(1, xb13)   # bottomup attends pcm13

                # expand {a13, bp13, a17, bp17} from [32,2] to [64]
                bncoef = sm.tile([32, 2, 4], f32, tag="bncoef", name="bncoef")
                nc.vector.tensor_copy(bncoef[:, :, 0], a13)
                nc.vector.tensor_copy(bncoef[:, :, 1], bp13)
                nc.vector.tensor_copy(bncoef[:, :, 2], a17)
                nc.vector.tensor_copy(bncoef[:, :, 3], bp17)
                psE = pse.tile([64, 4], f32, tag="psE", name="psE")
                nc.tensor.matmul(psE, whsb[:, 0, :], bncoef[:, 0, :],
                                 start=True, stop=False)
                nc.tensor.matmul(psE, whsb[:, 1, :], bncoef[:, 1, :],
                                 start=False, stop=True)
                E = t32("E", (64, 4))
                nc.scalar.activation(E, psE, AF.Copy)

                # coefs per channel: co[c, (b, j)]: j in {A13', A17', D'}
                co = sm.tile([64, 12], f32, tag="co", name="co")
                covw = co.rearrange("c (b j) -> c b j", j=3)
                nc.vector.tensor_scalar(covw[:, :, 0], wei_td, E[:, 0:1],
                                        -1.0, OP.mult, OP.mult)
                nc.vector.tensor_scalar(covw[:, :, 1], wei_bu, E[:, 2:3],
                                        -1.0, OP.mult, OP.mult)
                dt_ = t32("dt_", (64, 4))
                nc.vector.tensor_scalar(dt_, wei_td, E[:, 1:2], None, OP.mult)
                nc.vector.scalar_tensor_tensor(
                    out=covw[:, :, 2], in0=wei_bu, scalar=E[:, 3:4], in1=dt_,
                    op0=OP.mult, op1=OP.add,
                )

                # gather to this core's 128 partitions, then select batch
                psC = pse.tile([128, 12], f32, tag="psC", name="psC")
                nc.tensor.matmul(psC, selsb, co, start=True, stop=True)
                cf12 = t32("cf12", (128, 12))
                nc.scalar.activation(cf12, psC, AF.Copy)
                cfm = t32("cfm", (128, 12))
                nc.vector.tensor_mul(cfm, cf12, bmsb)
                cf = t32("cf", (128, 3))
                nc.vector.tensor_reduce(
                    out=cf,
                    in_=cfm.rearrange("p (b j) -> p j b", j=3),
                    axis=AX.X, op=OP.add,
                )

            # ---------------- apply ----------------
            with tc.tile_pool(name="io", bufs=2) as io:
                for ch in range(NCH):
                    t13 = io.tile([128, QR, CW], bf, tag="t13", name="t13")
                    t17 = io.tile([128, QR, CW], bf, tag="t17", name="t17")
                    nc.sync.dma_start(out=t13, in_=m13d[ch])
                    nc.sync.dma_start(out=t17, in_=m17d[ch])
                    u = io.tile([128, QR, CW], bf, tag="u", name="u")
                    nc.scalar.activation(
                        u, t13, AF.Identity,
                        scale=cf[:, 0:1], bias=cf[:, 2:3],
                    )
                    o = io.tile([128, QR, CW], bf, tag="o", name="o")
                    nc.vector.scalar_tensor_tensor(
                        out=o, in0=t17, scalar=cf[:, 1:2], in1=u,
                        op0=OP.mult, op1=OP.add,
                    )
                    nc.sync.dma_start(out=out_t[ch], in_=o)
    return nc


def _fused_inputs(cen, bn1_g, bn1_b, bn2_g, bn2_b,
                  td_w1, td_g1, td_be1, td_w2, td_g2, td_be2,
                  bu_w1, bu_g1, bu_be1, bu_w2, bu_g2, bu_be2):
    """Build the per-core input dicts for the fused NEFF."""
    f = np.float32
    shards = _shards_from_cen(cen)
    eyes = _eyes()

    wse1 = np.zeros((4, 32, 32), BF16)
    wse2 = np.zeros((2, 32, 64), BF16)
    for se, (w1, w2) in enumerate(((td_w1, td_w2), (bu_w1, bu_w2))):
        w1 = np.asarray(w1, f)       # [32, 64]
        w2 = np.asarray(w2, f)       # [64, 32]
        for h in (0, 1):
            wse1[2 * se + h] = w1[:, 32 * h : 32 * h + 32].T.astype(BF16)
        wse2[se] = w2.T.astype(BF16)

    whalf = np.zeros((2, 32, 64), f)
    for h in (0, 1):
        whalf[h, np.arange(32), np.arange(32) + 32 * h] = 1.0

    seg1 = np.zeros((2, 32, 2), f)
    seg1[0, :, 0], seg1[0, :, 1] = td_g1, td_be1
    seg1[1, :, 0], seg1[1, :, 1] = bu_g1, bu_be1
    seg2 = np.zeros((2, 64, 2), f)
    seg2[0, :, 0], seg2[0, :, 1] = td_g2, td_be2
    seg2[1, :, 0], seg2[1, :, 1] = bu_g2, bu_be2

    bnw = np.zeros((32, 2, 4), f)
    for h in (0, 1):
        c = np.arange(32) + 32 * h
        bnw[:, h, 0] = np.asarray(bn1_g, f)[c]
        bnw[:, h, 1] = np.asarray(bn1_b, f)[c]
        bnw[:, h, 2] = np.asarray(bn2_g, f)[c]
        bnw[:, h, 3] = np.asarray(bn2_b, f)[c]

    in_maps = []
    for k in range(NCORES):
        selk = np.zeros((64, 128), f)
        p = np.arange(128)
        cp = 32 * (k % 2) + p // 4
        selk[cp, p] = 1.0
        bmk = np.zeros((128, 4, 3), f)
        bmk[:, k // 2, :] = 1.0
        in_maps.append({
            "cenQ": shards[k], "eyes": eyes,
            "wse1": wse1, "wse2": wse2, "whalf": whalf,
            "seg1": seg1, "seg2": seg2, "bnw": bnw,
            "sel": selk, "bmask": bmk.reshape(128, 12),
        })
    return in_maps


def _shards_from_cen(cen):
    """Per-core bf16 chunk-major quarter shards cenQ: [NCH, 128, SR, CWF]."""
    cwf = CW + 2 * CHALO
    pl = np.ascontiguousarray(cen.reshape(P, H, W)).astype(BF16)
    rows = (np.arange(-RHALO, QR + RHALO)[None, :] + QR * np.arange(NQ)[:, None]) % H
    cols = (np.arange(-CHALO, CW + CHALO)[None, :] + CW * np.arange(NCH)[:, None]) % W
    shards = []
    for k in range(NCORES):
        sub = pl[32 * k : 32 * (k + 1)]          # [32, H, W]
        q = sub[:, rows, :]                      # [32, NQ, SR, W]
        q = q[:, :, :, cols]                     # [32, NQ, SR, NCH, CWF]
        q = q.transpose(3, 0, 1, 2, 4)           # [NCH, 32, NQ, SR, CWF]
        shards.append(np.ascontiguousarray(q.reshape(NCH, 128, SR, cwf)))
    return shards


def _eyes():
    e = np.zeros((2, 128, 128), BF16)
    e[0] = np.eye(128, dtype=np.float32)
    e[1] = -np.eye(128, dtype=np.float32)
    return e


def _host_glue(stats_list, bn1_g, bn1_b, bn2_g, bn2_b,
               td_w1, td_b1, td_g1, td_be1, td_w2, td_b2, td_g2, td_be2,
               bu_w1, bu_b1, bu_g1, bu_be1, bu_w2, bu_b2, bu_g2, bu_be2):
    """Combine per-core stats, run BN + SE exactly, return per-core coefs.

    stats_list[k]: [128, 4*NCH] f32, partitions = (plane_local, quarter),
    slots per chunk: [sumM13, sqM13, sumM17, sqM17].  pcm = -M.
    """
    f8 = np.float64
    per_plane = np.zeros((P, 4), f8)
    for k, st in enumerate(stats_list):
        t = st.astype(f8).reshape(PPC, NQ, NCH, 4).sum(axis=(1, 2))
        per_plane[32 * k : 32 * (k + 1)] += t
    sum13 = -per_plane[:, 0].reshape(B, C)   # sum(pcm13)
    sq13 = per_plane[:, 1].reshape(B, C)
    sum17 = -per_plane[:, 2].reshape(B, C)
    sq17 = per_plane[:, 3].reshape(B, C)

    n = B * H * W
    n_sq = B * H * W // SSTEP

    def bn_affine(sm, sq, g, b):
        mean = sm.sum(0) / n
        var = sq.sum(0) / n_sq - mean * mean
        a = g.astype(f8) / np.sqrt(var + EPS)
        return a, b.astype(f8) - mean * a

    a1, b1 = bn_affine(sum13, sq13, bn1_g, bn1_b)   # BN for pcm13
    a2, b2 = bn_affine(sum17, sq17, bn2_g, bn2_b)   # BN for pcm17

    p13 = a1[None, :] * (sum13 / (H * W)) + b1[None, :]
    p17 = a2[None, :] * (sum17 / (H * W)) + b2[None, :]

    def se(p, w1, bb1, g1, be1, w2, bb2, g2, be2):
        y = p @ w1.astype(f8).T + bb1.astype(f8)[None, :]
        mu, v = y.mean(0), y.var(0)
        y = (y - mu) / np.sqrt(v + EPS) * g1.astype(f8) + be1.astype(f8)
        y = np.maximum(y, 0.0)
        z = y @ w2.astype(f8).T + bb2.astype(f8)[None, :]
        mu, v = z.mean(0), z.var(0)
        z = (z - mu) / np.sqrt(v + EPS) * g2.astype(f8) + be2.astype(f8)
        return 1.0 / (1.0 + np.exp(-z))

    td_wei = se(p17, td_w1, td_b1, td_g1, td_be1, td_w2, td_b2, td_g2, td_be2)
    bu_wei = se(p13, bu_w1, bu_b1, bu_g1, bu_be1, bu_w2, bu_b2, bu_g2, bu_be2)

    # out = td*pcm13N + bu*pcm17N = A13*pcm13 + A17*pcm17 + D, pcm = -M:
    A13p = -(td_wei * a1[None, :]).reshape(P)
    A17p = -(bu_wei * a2[None, :]).reshape(P)
    Dc = (td_wei * b1[None, :] + bu_wei * b2[None, :]).reshape(P)

    coefs = []
    for k in range(NCORES):
        cf = np.zeros((PPC, NQ, 4), np.float32)
        cf[:, :, 0] = A13p[32 * k : 32 * (k + 1), None]
        cf[:, :, 1] = A17p[32 * k : 32 * (k + 1), None]
        cf[:, :, 2] = Dc[32 * k : 32 * (k + 1), None]
        coefs.append(cf.reshape(128, 4))
    return coefs


def _run(nc, in_maps, trace=False):
    from concourse.bass_utils import run_bass_kernel_spmd

    return run_bass_kernel_spmd(nc, in_maps, list(range(NCORES)), trace=trace)


def _get_kernels():
    if "pass_a" not in _cache:
        nca = _build_pass_a()
        nca.compile()
        _cache["pass_a"] = nca
    if "pass_b" not in _cache:
        ncb = _build_pass_b()
        ncb.compile()
        _cache["pass_b"] = ncb
    return _cache["pass_a"], _cache["pass_b"]


def _assemble_out(res_b):
    out = np.empty((P, H, W), np.float32)
    for k in range(NCORES):
        q = np.asarray(res_b[k]["out"]).astype(np.float32)   # [NCH,128,QR,CW]
        q = q.reshape(NCH, PPC, NQ, QR, CW).transpose(1, 2, 3, 0, 4)
        out[32 * k : 32 * (k + 1)] = q.reshape(PPC, H, W)
    return out.reshape(B, C, H, W)


FUSED = os.environ.get("FUSED", "0") == "1"


def _get_fused():
    if "fused" not in _cache:
        ncf = _build_fused()
        ncf.compile()
        _cache["fused"] = ncf
    return _cache["fused"]


def kernel(cen, bn1_g, bn1_b, bn2_g, bn2_b,
           td_w1, td_b1, td_g1, td_be1, td_w2, td_b2, td_g2, td_be2,
           bu_w1, bu_b1, bu_g1, bu_be1, bu_w2, bu_b2, bu_g2, bu_be2):
    cen = np.asarray(cen, np.float32)
    if FUSED:
        ncf = _get_fused()
        in_maps = _fused_inputs(
            cen, bn1_g, bn1_b, bn2_g, bn2_b,
            td_w1, td_g1, td_be1, td_w2, td_g2, td_be2,
            bu_w1, bu_g1, bu_be1, bu_w2, bu_g2, bu_be2,
        )
        res = _run(ncf, in_maps).results
        return _assemble_out(res)
    nca, ncb = _get_kernels()

    shards = _shards_from_cen(cen)
    eyes = _eyes()
    res_a = _run(nca, [{"cenQ": e, "eyes": eyes} for e in shards]).results

    coefs = _host_glue(
        [r["stats"] for r in res_a],
        bn1_g, bn1_b, bn2_g, bn2_b,
        td_w1, td_b1, td_g1, td_be1, td_w2, td_b2, td_g2, td_be2,
        bu_w1, bu_b1, bu_g1, bu_be1, bu_w2, bu_b2, bu_g2, bu_be2,
    )

    in_b = [
        {"m13": r["m13"], "m17": r["m17"], "coef": cf}
        for r, cf in zip(res_a, coefs)
    ]
    res_b = _run(ncb, in_b).results
    return _assemble_out(res_b)

